# revision 12
# baseline (speedup 1.0000x reference)
"""Trainium2 Bass kernel for CropConv: 3x3 same-padding conv (64->64 ch) on
[16, 64, 128, 128] fp32 input, with a static crop mask zeroing output rows/cols
[44:84).

Strategy (data-parallel over batch, 8 cores x 2 images each):
  - Host marshals x into a zero-padded row-major layout with row stride 129
    (131 padded rows; the left zero column of each row doubles as the previous
    row's right pad), so every conv tap (kh, kw) of an output row-chunk is one
    contiguous rhs slice.
  - Per core, image 0 lives in SBUF partitions 0-63 (partition = in-channel),
    image 1 in partitions 64-127.
  - The conv is 9 PSUM-accumulated TensorE matmuls per output chunk:
    out[oc, pix] += W[kh,kw][ic, oc].T @ x[ic, shifted pix].  K = M = 64, so
    four matmuls run concurrently in the four 64x64 quadrants of the PE array
    (row-half = image, col-half = adjacent chunk pairing (2p, 2p+1)).
    Two pairs share each tap loop so consecutive matmuls reuse weights.
  - DMA count is minimized (each dma_start costs ~0.6-1us serialized on its
    HWDGE ring): one weight load, six x segments sized so early rows land
    first, stores batched 3 pairs (294 KB) at a time.  Loads/stores alternate
    between the sync and scalar rings.
  - PSUM is evicted fp32 -> fp16 (vector + scalar-ACT engines, crop mask
    fused as a multiply on the masked row range) into a chunk-major SBUF
    stage; the last store batch folds in the leftover chunk 42 via a padded
    44-chunk output layout.  The host untangles and upcasts to fp32.
"""

import numpy as np

# ---- problem constants (hardcoded; kernel.py must be self-contained) ----
B, C, H, W = 16, 64, 128, 128
OC, KS = 64, 3
N_CORES = 8
IMGS = B // N_CORES  # 2 images per core

WP = W + 1            # padded row stride: 129
HP = H + 3            # padded rows in the x buffer: 131
XLEN = HP * WP        # 16899 elems per partition

RPC = 3               # output rows per chunk
NCH = (H + RPC - 1) // RPC   # 43 chunks per image (last has 2 rows)
NCHP = NCH + 1        # padded to 44 (chunk 43 is garbage, dropped on host)
NPAIR = 21            # adjacent-chunk pairs (2p, 2p+1); chunk 42 leftover
NSLOT = 22            # stage slots per image: 21 pairs + leftover
CHN = RPC * WP        # matmul free dim per full chunk: 387
CHS = RPC * W         # compact stage slot stride: 384
STLEN = 2 * NSLOT * CHS   # stage free size: 16896

CROP0, CROP1 = 44, 84  # masked rows/cols [44, 84)

_CACHE = {}


def _build_module():
    import concourse.tile as tile
    from concourse import bacc, mybir

    f32 = mybir.dt.float32
    f16 = mybir.dt.float16
    bf16 = mybir.dt.bfloat16

    nc = bacc.Bacc("TRN2", target_bir_lowering=False, debug=False,
                   num_devices=N_CORES)

    x_ap = nc.dram_tensor("xin", [IMGS, C, XLEN], bf16,
                          kind="ExternalInput").ap()
    # weights pre-duplicated on host into both partition halves
    w_ap = nc.dram_tensor("wt", [2 * C, KS * KS * OC], bf16,
                          kind="ExternalInput").ap()
    m_ap = nc.dram_tensor("mk", [128, 2 * CHS], f32,
                          kind="ExternalInput").ap()
    # chunk-major output: [img, chunk, oc, 3*128]; host untangles
    y_ap = nc.dram_tensor("yout", [IMGS, NCHP, OC, CHS], f16,
                          kind="ExternalOutput").ap()

    x_bc = x_ap.rearrange("b c l -> (b c) l")  # [128, XLEN]

    with tile.TileContext(nc) as tc:
        with tc.tile_pool(name="big", bufs=1) as big, \
             tc.tile_pool(name="psum", bufs=8, space="PSUM") as pp:

            x_sb = big.tile([128, XLEN], bf16, tag="xbuf")
            stage = big.tile([128, STLEN], f16, tag="stage")
            w_sb = big.tile([128, KS * KS * OC], bf16, tag="wbuf")
            mk_sb = big.tile([128, 2 * CHS], f32, tag="mask")

            # sync ring: weights, then x rows 0-8 (the first compute dep),
            # then big mid segments.  scalar ring: x rows 8-24 next, etc.
            # Segment sizes keep early rows arriving well ahead of the
            # matmul stream while minimizing dma_start count.
            nc.sync.dma_start(out=w_sb, in_=w_ap)
            segs = [(0, 8, nc.sync), (8, 24, nc.scalar), (24, 56, nc.sync),
                    (56, 88, nc.scalar), (88, 120, nc.sync),
                    (120, HP, nc.scalar)]
            for (a, b_, eng) in segs:
                eng.dma_start(out=x_sb[:, a * WP:b_ * WP],
                              in_=x_bc[:, a * WP:b_ * WP])
            # mask arrives well before pair 7 needs it
            nc.scalar.dma_start(out=mk_sb, in_=m_ap)

            def lhsT(half, t):
                return w_sb[half * 64:(half + 1) * 64, t * OC:(t + 1) * OC]

            def rhs(half, c, kh, kw, n):
                off = (RPC * c + kh) * WP + kw
                return x_sb[half * 64:(half + 1) * 64, off:off + n]

            TAPS = [(kh, kw) for kh in range(KS) for kw in range(KS)]

            mk3 = mk_sb.rearrange("p (m h w) -> p m h w", m=2, w=W)

            def slot(i, p):
                return (i * NSLOT + p) * CHS

            def evict(p, bank, i):
                """PSUM bank (img i, pair p) -> fp16 stage, mask fused."""
                src = bank[:, 0:CHN].rearrange(
                    "p (h w) -> p h w", w=WP)[:, :, 0:W]
                dst = stage[:, slot(i, p):slot(i, p) + CHS].rearrange(
                    "p (h w) -> p h w", w=W)
                if p == 7:            # chunks (14,15): rows 44-47 masked
                    nc.vector.tensor_mul(dst, src, mk3[:, 0])
                elif 8 <= p <= 13:    # chunks (16..27): rows 48-83 masked
                    nc.vector.tensor_mul(dst, src, mk3[:, 1])
                elif i == 0:
                    nc.vector.tensor_copy(dst, src)
                else:
                    nc.scalar.copy(dst, src)

            def store_batch(s0, np_):
                """np_ pair-slots starting at slot s0, one DMA per image."""
                for i, eng in ((0, nc.sync), (1, nc.scalar)):
                    src = stage[:, slot(i, s0):slot(i, s0) + np_ * CHS]
                    dst = y_ap[i, 2 * s0:2 * s0 + 2 * np_, :, :].rearrange(
                        "(pr par) o f -> (par o) pr f", par=2)
                    eng.dma_start(out=dst,
                                  in_=src.rearrange("p (pr f) -> p pr f",
                                                    f=CHS))

            # PE warm-up: dummy matmuls on scratch SBUF (stage slot written
            # only much later) keep the PE busy through the HAM activity
            # window during the initial x-load wait.  The HAM clock gate
            # only un-throttles after one *fully busy* ~3.4us window, so the
            # dummies must bridge gaplessly into the real matmul stream:
            # 50 x N=128 (~107ns each cold) slightly overshoots the expected
            # seg0-completion time.
            dum = pp.tile([128, 512], f32, tag="ps", name="dum")
            scr = stage[0:64, slot(1, 20):slot(1, 20) + 512]
            for _ in range(50):
                nc.tensor.matmul(dum[0:64, 0:128], scr[:, 0:64],
                                 scr[:, 0:128], start=True, stop=True,
                                 skip_group_check=True)

            # matmul groups of 2 pairs (pair 0 alone so the stream can start
            # on the first 8-row x segment): consecutive matmuls within a
            # tap share the stationary weights
            groups = [(0,)] + [(2 * g + 1, 2 * g + 2) for g in range(10)]
            for grp in groups:
                banks = {}
                for p in grp:
                    banks[p] = (pp.tile([128, 512], f32, tag="ps",
                                        name=f"pa{p}"),
                                pp.tile([128, 512], f32, tag="ps",
                                        name=f"pb{p}"))
                for t, (kh, kw) in enumerate(TAPS):
                    st, sp = (t == 0), (t == len(TAPS) - 1)
                    for half in (0, 1):   # img half: same lhsT across pairs
                        for p in grp:
                            bank = banks[p][half]
                            for c_par in (0, 1):
                                nc.tensor.matmul(
                                    bank[c_par * 64:(c_par + 1) * 64, 0:CHN],
                                    lhsT(half, t),
                                    rhs(half, 2 * p + c_par, kh, kw, CHN),
                                    start=st, stop=sp, skip_group_check=True)
                for p in grp:
                    evict(p, banks[p][0], 0)
                    evict(p, banks[p][1], 1)
                    if p in (3, 7, 11, 15):
                        store_batch(p - 3, 4)
                    elif p == 18:
                        store_batch(16, 3)

            # leftover chunk 42 (rows 126-127): img0 in quadrant (r0, c0),
            # img1 in quadrant (r1, c0) so both land on PSUM partitions 0-63
            # of their own bank and evict partition-aligned into slot 21
            n2 = 2 * WP  # 258
            pc_ = pp.tile([128, 512], f32, tag="ps", name="pc_")
            pd_ = pp.tile([128, 512], f32, tag="ps", name="pd_")
            for t, (kh, kw) in enumerate(TAPS):
                st, sp = (t == 0), (t == len(TAPS) - 1)
                nc.tensor.matmul(pc_[0:64, 0:n2], lhsT(0, t),
                                 rhs(0, NCH - 1, kh, kw, n2), start=st,
                                 stop=sp, skip_group_check=True)
                nc.tensor.matmul(pd_[0:64, 0:n2], lhsT(1, t),
                                 rhs(1, NCH - 1, kh, kw, n2), start=st,
                                 stop=sp, skip_group_check=True)
            for i, bank, eng in ((0, pc_, nc.vector), (1, pd_, nc.scalar)):
                src = bank[0:64, 0:n2].rearrange(
                    "p (h w) -> p h w", w=WP)[:, :, 0:W]
                dst = stage[0:64, slot(i, 21):slot(i, 21) + 2 * W].rearrange(
                    "p (h w) -> p h w", w=W)
                if i == 0:
                    eng.tensor_copy(dst, src)
                else:
                    eng.copy(dst, src)
            # final batch: pairs 19-20 + leftover slot (chunks 38-42 + pad)
            store_batch(19, 3)

    nc.compile()
    return nc


def _get_module():
    if "nc" not in _CACHE:
        _CACHE["nc"] = _build_module()
    return _CACHE["nc"]


def _build_mask():
    """[128, 768] fp32: [:, 0:384] = pair-7 mask (chunk 14 row 44 only in
    partitions 0-63, chunk 15 rows 45-47 in partitions 64-127); [:, 384:768]
    = full mask (all three rows) for pairs 8..13 (chunks 16..27)."""
    mk = np.ones((128, 2, RPC, W), dtype=np.float32)
    mk[:, 1, :, CROP0:CROP1] = 0.0          # full mask: every row
    mk[0:64, 0, 2, CROP0:CROP1] = 0.0       # pair 7, chunk 14: row 44 (j=2)
    mk[64:128, 0, :, CROP0:CROP1] = 0.0     # pair 7, chunk 15: rows 45-47
    return mk.reshape(128, 2 * CHS)


def _make_in_maps(x, weight):
    x = np.asarray(x, dtype=np.float32)
    weight = np.asarray(weight, dtype=np.float32)
    # host marshaling: pad x into the row-major stride-129 layout
    xp = np.zeros((B, C, HP, WP), dtype=np.float32)
    xp[:, :, 1:H + 1, 1:W + 1] = x
    xp = xp.reshape(B, C, XLEN)
    import ml_dtypes
    xp = xp.astype(ml_dtypes.bfloat16)
    # weight [oc, ic, kh, kw] -> [ic, (kh kw), oc], duplicated in both halves
    wt = np.ascontiguousarray(
        weight.transpose(1, 2, 3, 0).reshape(C, KS * KS * OC)
    ).astype(ml_dtypes.bfloat16)
    wt = np.concatenate([wt, wt], axis=0)  # [128, 576]
    mk = _build_mask()
    return [
        {"xin": np.ascontiguousarray(xp[k * IMGS:(k + 1) * IMGS]), "wt": wt,
         "mk": mk}
        for k in range(N_CORES)
    ]


def kernel(x, weight):
    from concourse.bass_utils import run_bass_kernel_spmd

    nc = _get_module()
    in_maps = _make_in_maps(x, weight)
    res = run_bass_kernel_spmd(nc, in_maps, list(range(N_CORES)))
    # host unshard: [2, 44, 64, 384] fp16 chunk-major -> [2, 64, 128, 128]
    outs = []
    for k in range(N_CORES):
        y = np.asarray(res.results[k]["yout"])  # [IMGS, NCHP, OC, CHS] fp16
        y = y.reshape(IMGS, NCHP, OC, RPC, W).transpose(0, 2, 1, 3, 4)
        y = y.reshape(IMGS, OC, NCHP * RPC, W)[:, :, :H, :]
        outs.append(y.astype(np.float32))
    return np.concatenate(outs, axis=0)


# revision 13
# speedup vs baseline: 1.0177x; 1.0177x over previous
"""Trainium2 Bass kernel for CropConv: 3x3 same-padding conv (64->64 ch) on
[16, 64, 128, 128] fp32 input, with a static crop mask zeroing output rows/cols
[44:84).

Strategy (data-parallel over batch, 8 cores x 2 images each):
  - Host marshals x into a zero-padded row-major layout with row stride 129
    (131 padded rows; the left zero column of each row doubles as the previous
    row's right pad), so every conv tap (kh, kw) of an output row-chunk is one
    contiguous rhs slice.
  - Per core, image 0 lives in SBUF partitions 0-63 (partition = in-channel),
    image 1 in partitions 64-127.
  - The conv is 9 PSUM-accumulated TensorE matmuls per output chunk:
    out[oc, pix] += W[kh,kw][ic, oc].T @ x[ic, shifted pix].  K = M = 64, so
    four matmuls run concurrently in the four 64x64 quadrants of the PE array
    (row-half = image, col-half = adjacent chunk pairing (2p, 2p+1)).
    Two pairs share each tap loop so consecutive matmuls reuse weights.
  - DMA count is minimized (each dma_start costs ~0.6-1us serialized on its
    HWDGE ring): one weight load, six x segments sized so early rows land
    first, stores batched 3 pairs (294 KB) at a time.  Loads/stores alternate
    between the sync and scalar rings.
  - PSUM is evicted fp32 -> fp16 (vector + scalar-ACT engines, crop mask
    fused as a multiply on the masked row range) into a chunk-major SBUF
    stage; the last store batch folds in the leftover chunk 42 via a padded
    44-chunk output layout.  The host untangles and upcasts to fp32.
"""

import numpy as np

# ---- problem constants (hardcoded; kernel.py must be self-contained) ----
B, C, H, W = 16, 64, 128, 128
OC, KS = 64, 3
N_CORES = 8
IMGS = B // N_CORES  # 2 images per core

WP = W + 1            # padded row stride: 129
HP = H + 3            # padded rows in the x buffer: 131
XLEN = HP * WP        # 16899 elems per partition

RPC = 3               # output rows per chunk
NCH = (H + RPC - 1) // RPC   # 43 chunks per image (last has 2 rows)
NCHP = NCH + 1        # padded to 44 (chunk 43 is garbage, dropped on host)
NPAIR = 21            # adjacent-chunk pairs (2p, 2p+1); chunk 42 leftover
NSLOT = 22            # stage slots per image: 21 pairs + leftover
CHN = RPC * WP        # matmul free dim per full chunk: 387
CHS = RPC * W         # compact stage slot stride: 384
STLEN = 2 * NSLOT * CHS   # stage free size: 16896

CROP0, CROP1 = 44, 84  # masked rows/cols [44, 84)

_CACHE = {}


def _build_module():
    import concourse.tile as tile
    from concourse import bacc, mybir

    f32 = mybir.dt.float32
    f16 = mybir.dt.float16
    bf16 = mybir.dt.bfloat16

    nc = bacc.Bacc("TRN2", target_bir_lowering=False, debug=False,
                   num_devices=N_CORES)

    x_ap = nc.dram_tensor("xin", [IMGS, C, XLEN], bf16,
                          kind="ExternalInput").ap()
    # weights pre-duplicated on host into both partition halves
    w_ap = nc.dram_tensor("wt", [2 * C, KS * KS * OC], bf16,
                          kind="ExternalInput").ap()
    m_ap = nc.dram_tensor("mk", [128, 2 * CHS], f32,
                          kind="ExternalInput").ap()
    # chunk-major output: [img, chunk, oc, 3*128]; host untangles
    y_ap = nc.dram_tensor("yout", [IMGS, NCHP, OC, CHS], f16,
                          kind="ExternalOutput").ap()

    x_bc = x_ap.rearrange("b c l -> (b c) l")  # [128, XLEN]

    with tile.TileContext(nc) as tc:
        with tc.tile_pool(name="big", bufs=1) as big, \
             tc.tile_pool(name="psum", bufs=8, space="PSUM") as pp:

            x_sb = big.tile([128, XLEN], bf16, tag="xbuf")
            stage = big.tile([128, STLEN], f16, tag="stage")
            w_sb = big.tile([128, KS * KS * OC], bf16, tag="wbuf")
            mk_sb = big.tile([128, 2 * CHS], f32, tag="mask")

            # sync ring: weights, then x rows 0-8 (the first compute dep),
            # then big mid segments.  scalar ring: x rows 8-24 next, etc.
            # Segment sizes keep early rows arriving well ahead of the
            # matmul stream while minimizing dma_start count.
            nc.sync.dma_start(out=w_sb, in_=w_ap)
            segs = [(0, 8, nc.sync), (8, 24, nc.scalar), (24, 56, nc.sync),
                    (56, 88, nc.scalar), (88, 120, nc.sync),
                    (120, HP, nc.scalar)]
            for (a, b_, eng) in segs:
                eng.dma_start(out=x_sb[:, a * WP:b_ * WP],
                              in_=x_bc[:, a * WP:b_ * WP])
            # mask arrives well before pair 7 needs it
            nc.scalar.dma_start(out=mk_sb, in_=m_ap)

            def lhsT(half, t):
                return w_sb[half * 64:(half + 1) * 64, t * OC:(t + 1) * OC]

            def rhs(half, c, kh, kw, n):
                off = (RPC * c + kh) * WP + kw
                return x_sb[half * 64:(half + 1) * 64, off:off + n]

            TAPS = [(kh, kw) for kh in range(KS) for kw in range(KS)]

            mk3 = mk_sb.rearrange("p (m h w) -> p m h w", m=2, w=W)

            def slot(i, p):
                return (i * NSLOT + p) * CHS

            def evict(p, bank, i):
                """PSUM bank (img i, pair p) -> fp16 stage, mask fused."""
                src = bank[:, 0:CHN].rearrange(
                    "p (h w) -> p h w", w=WP)[:, :, 0:W]
                dst = stage[:, slot(i, p):slot(i, p) + CHS].rearrange(
                    "p (h w) -> p h w", w=W)
                if p == 7:            # chunks (14,15): rows 44-47 masked
                    nc.vector.tensor_mul(dst, src, mk3[:, 0])
                elif 8 <= p <= 13:    # chunks (16..27): rows 48-83 masked
                    nc.vector.tensor_mul(dst, src, mk3[:, 1])
                elif i == 0:
                    nc.vector.tensor_copy(dst, src)
                else:
                    nc.scalar.copy(dst, src)

            def store_batch(s0, np_):
                """np_ pair-slots starting at slot s0, one DMA per image."""
                for i, eng in ((0, nc.sync), (1, nc.scalar)):
                    src = stage[:, slot(i, s0):slot(i, s0) + np_ * CHS]
                    dst = y_ap[i, 2 * s0:2 * s0 + 2 * np_, :, :].rearrange(
                        "(pr par) o f -> (par o) pr f", par=2)
                    eng.dma_start(out=dst,
                                  in_=src.rearrange("p (pr f) -> p pr f",
                                                    f=CHS))

            # PE warm-up: full-width (128x128) dummy matmuls on scratch SBUF
            # (stage slot written only much later) during the initial x-load
            # wait.  The HAM clock gate un-throttles only after a ~3.4us
            # window of *high-utilization* PE activity (64x64 single-quadrant
            # dummies measurably never flip it), and once warm it re-throttles
            # only after a fully idle window, so the dummies may end a couple
            # of microseconds before the real stream starts.
            dum = pp.tile([128, 512], f32, tag="ps", name="dum")
            scr = stage[:, slot(1, 20):slot(1, 20) + 512]
            for _ in range(40):
                nc.tensor.matmul(dum[:, 0:128], scr[:, 0:128],
                                 scr[:, 0:128], start=True, stop=True,
                                 skip_group_check=True)

            # matmul groups of 2 pairs (pair 0 alone so the stream can start
            # on the first 8-row x segment): consecutive matmuls within a
            # tap share the stationary weights
            groups = [(0,)] + [(2 * g + 1, 2 * g + 2) for g in range(10)]
            for grp in groups:
                banks = {}
                for p in grp:
                    banks[p] = (pp.tile([128, 512], f32, tag="ps",
                                        name=f"pa{p}"),
                                pp.tile([128, 512], f32, tag="ps",
                                        name=f"pb{p}"))
                for t, (kh, kw) in enumerate(TAPS):
                    st, sp = (t == 0), (t == len(TAPS) - 1)
                    for half in (0, 1):   # img half: same lhsT across pairs
                        for p in grp:
                            bank = banks[p][half]
                            for c_par in (0, 1):
                                nc.tensor.matmul(
                                    bank[c_par * 64:(c_par + 1) * 64, 0:CHN],
                                    lhsT(half, t),
                                    rhs(half, 2 * p + c_par, kh, kw, CHN),
                                    start=st, stop=sp, skip_group_check=True)
                for p in grp:
                    evict(p, banks[p][0], 0)
                    evict(p, banks[p][1], 1)
                    if p in (3, 7, 11, 15):
                        store_batch(p - 3, 4)
                    elif p == 18:
                        store_batch(16, 3)

            # leftover chunk 42 (rows 126-127): img0 in quadrant (r0, c0),
            # img1 in quadrant (r1, c0) so both land on PSUM partitions 0-63
            # of their own bank and evict partition-aligned into slot 21
            n2 = 2 * WP  # 258
            pc_ = pp.tile([128, 512], f32, tag="ps", name="pc_")
            pd_ = pp.tile([128, 512], f32, tag="ps", name="pd_")
            for t, (kh, kw) in enumerate(TAPS):
                st, sp = (t == 0), (t == len(TAPS) - 1)
                nc.tensor.matmul(pc_[0:64, 0:n2], lhsT(0, t),
                                 rhs(0, NCH - 1, kh, kw, n2), start=st,
                                 stop=sp, skip_group_check=True)
                nc.tensor.matmul(pd_[0:64, 0:n2], lhsT(1, t),
                                 rhs(1, NCH - 1, kh, kw, n2), start=st,
                                 stop=sp, skip_group_check=True)
            for i, bank, eng in ((0, pc_, nc.vector), (1, pd_, nc.scalar)):
                src = bank[0:64, 0:n2].rearrange(
                    "p (h w) -> p h w", w=WP)[:, :, 0:W]
                dst = stage[0:64, slot(i, 21):slot(i, 21) + 2 * W].rearrange(
                    "p (h w) -> p h w", w=W)
                if i == 0:
                    eng.tensor_copy(dst, src)
                else:
                    eng.copy(dst, src)
            # final batch: pairs 19-20 + leftover slot (chunks 38-42 + pad)
            store_batch(19, 3)

    nc.compile()
    return nc


def _get_module():
    if "nc" not in _CACHE:
        _CACHE["nc"] = _build_module()
    return _CACHE["nc"]


def _build_mask():
    """[128, 768] fp32: [:, 0:384] = pair-7 mask (chunk 14 row 44 only in
    partitions 0-63, chunk 15 rows 45-47 in partitions 64-127); [:, 384:768]
    = full mask (all three rows) for pairs 8..13 (chunks 16..27)."""
    mk = np.ones((128, 2, RPC, W), dtype=np.float32)
    mk[:, 1, :, CROP0:CROP1] = 0.0          # full mask: every row
    mk[0:64, 0, 2, CROP0:CROP1] = 0.0       # pair 7, chunk 14: row 44 (j=2)
    mk[64:128, 0, :, CROP0:CROP1] = 0.0     # pair 7, chunk 15: rows 45-47
    return mk.reshape(128, 2 * CHS)


def _make_in_maps(x, weight):
    x = np.asarray(x, dtype=np.float32)
    weight = np.asarray(weight, dtype=np.float32)
    # host marshaling: pad x into the row-major stride-129 layout
    xp = np.zeros((B, C, HP, WP), dtype=np.float32)
    xp[:, :, 1:H + 1, 1:W + 1] = x
    xp = xp.reshape(B, C, XLEN)
    import ml_dtypes
    xp = xp.astype(ml_dtypes.bfloat16)
    # weight [oc, ic, kh, kw] -> [ic, (kh kw), oc], duplicated in both halves
    wt = np.ascontiguousarray(
        weight.transpose(1, 2, 3, 0).reshape(C, KS * KS * OC)
    ).astype(ml_dtypes.bfloat16)
    wt = np.concatenate([wt, wt], axis=0)  # [128, 576]
    mk = _build_mask()
    return [
        {"xin": np.ascontiguousarray(xp[k * IMGS:(k + 1) * IMGS]), "wt": wt,
         "mk": mk}
        for k in range(N_CORES)
    ]


def kernel(x, weight):
    from concourse.bass_utils import run_bass_kernel_spmd

    nc = _get_module()
    in_maps = _make_in_maps(x, weight)
    res = run_bass_kernel_spmd(nc, in_maps, list(range(N_CORES)))
    # host unshard: [2, 44, 64, 384] fp16 chunk-major -> [2, 64, 128, 128]
    outs = []
    for k in range(N_CORES):
        y = np.asarray(res.results[k]["yout"])  # [IMGS, NCHP, OC, CHS] fp16
        y = y.reshape(IMGS, NCHP, OC, RPC, W).transpose(0, 2, 1, 3, 4)
        y = y.reshape(IMGS, OC, NCHP * RPC, W)[:, :, :H, :]
        outs.append(y.astype(np.float32))
    return np.concatenate(outs, axis=0)


# revision 17
# speedup vs baseline: 1.0231x; 1.0053x over previous
"""Trainium2 Bass kernel for CropConv: 3x3 same-padding conv (64->64 ch) on
[16, 64, 128, 128] fp32 input, with a static crop mask zeroing output rows/cols
[44:84).

Strategy (data-parallel over batch, 8 cores x 2 images each):
  - Host marshals x into a zero-padded row-major layout with row stride 129
    (131 padded rows; the left zero column of each row doubles as the previous
    row's right pad), so every conv tap (kh, kw) of an output row-chunk is one
    contiguous rhs slice.
  - Per core, image 0 lives in SBUF partitions 0-63 (partition = in-channel),
    image 1 in partitions 64-127.
  - The conv is 9 PSUM-accumulated TensorE matmuls per output chunk:
    out[oc, pix] += W[kh,kw][ic, oc].T @ x[ic, shifted pix].  K = M = 64, so
    four matmuls run concurrently in the four 64x64 quadrants of the PE array
    (row-half = image, col-half = adjacent chunk pairing (2p, 2p+1)).
    Two pairs share each tap loop so consecutive matmuls reuse weights.
  - DMA count is minimized (each dma_start costs ~0.6-1us serialized on its
    HWDGE ring): one weight load, six x segments sized so early rows land
    first, stores batched 3 pairs (294 KB) at a time.  Loads/stores alternate
    between the sync and scalar rings.
  - PSUM is evicted fp32 -> fp16 (vector + scalar-ACT engines, crop mask
    fused as a multiply on the masked row range) into a chunk-major SBUF
    stage; the last store batch folds in the leftover chunk 42 via a padded
    44-chunk output layout.  The host untangles and upcasts to fp32.
"""

import numpy as np

# ---- problem constants (hardcoded; kernel.py must be self-contained) ----
B, C, H, W = 16, 64, 128, 128
OC, KS = 64, 3
N_CORES = 8
IMGS = B // N_CORES  # 2 images per core

WP = W + 1            # padded row stride: 129
HP = H + 3            # padded rows in the x buffer: 131
XLEN = HP * WP        # 16899 elems per partition

RPC = 3               # output rows per chunk
NCH = (H + RPC - 1) // RPC   # 43 chunks per image (last has 2 rows)
NCHP = NCH + 1        # padded to 44 (chunk 43 is garbage, dropped on host)
NPAIR = 21            # adjacent-chunk pairs (2p, 2p+1); chunk 42 leftover
NSLOT = 22            # stage slots per image: 21 pairs + leftover
CHN = RPC * WP        # matmul free dim per full chunk: 387
CHS = RPC * W         # compact stage slot stride: 384
STLEN = 2 * NSLOT * CHS   # stage free size: 16896

CROP0, CROP1 = 44, 84  # masked rows/cols [44, 84)

_CACHE = {}


def _build_module():
    import concourse.tile as tile
    from concourse import bacc, mybir

    f32 = mybir.dt.float32
    f16 = mybir.dt.float16
    bf16 = mybir.dt.bfloat16

    nc = bacc.Bacc("TRN2", target_bir_lowering=False, debug=False,
                   num_devices=N_CORES)

    x_ap = nc.dram_tensor("xin", [IMGS, C, XLEN], bf16,
                          kind="ExternalInput").ap()
    # weights pre-duplicated on host into both partition halves
    w_ap = nc.dram_tensor("wt", [2 * C, KS * KS * OC], bf16,
                          kind="ExternalInput").ap()
    m_ap = nc.dram_tensor("mk", [128, 2 * CHS], f32,
                          kind="ExternalInput").ap()
    # chunk-major output: [img, chunk, oc, 3*128]; host untangles
    y_ap = nc.dram_tensor("yout", [IMGS, NCHP, OC, CHS], f16,
                          kind="ExternalOutput").ap()

    x_bc = x_ap.rearrange("b c l -> (b c) l")  # [128, XLEN]

    with tile.TileContext(nc) as tc:
        with tc.tile_pool(name="big", bufs=1) as big, \
             tc.tile_pool(name="psum", bufs=8, space="PSUM") as pp:

            x_sb = big.tile([128, XLEN], bf16, tag="xbuf")
            stage = big.tile([128, STLEN], f16, tag="stage")
            w_sb = big.tile([128, KS * KS * OC], bf16, tag="wbuf")
            mk_sb = big.tile([128, 2 * CHS], f32, tag="mask")

            # sync ring: weights, then x rows 0-8 (the first compute dep),
            # then big mid segments.  scalar ring: x rows 8-24 next, etc.
            # Segment sizes keep early rows arriving well ahead of the
            # matmul stream while minimizing dma_start count.
            nc.sync.dma_start(out=x_sb[:, 0:8 * WP], in_=x_bc[:, 0:8 * WP])
            nc.sync.dma_start(out=w_sb, in_=w_ap)
            segs = [(8, 24, nc.scalar), (24, 56, nc.sync),
                    (56, 88, nc.scalar), (88, 120, nc.sync),
                    (120, HP, nc.scalar)]
            for (a, b_, eng) in segs:
                eng.dma_start(out=x_sb[:, a * WP:b_ * WP],
                              in_=x_bc[:, a * WP:b_ * WP])
            # mask arrives well before pair 7 needs it
            nc.scalar.dma_start(out=mk_sb, in_=m_ap)

            def lhsT(half, t):
                return w_sb[half * 64:(half + 1) * 64, t * OC:(t + 1) * OC]

            def rhs(half, c, kh, kw, n):
                off = (RPC * c + kh) * WP + kw
                return x_sb[half * 64:(half + 1) * 64, off:off + n]

            TAPS = [(kh, kw) for kh in range(KS) for kw in range(KS)]

            mk3 = mk_sb.rearrange("p (m h w) -> p m h w", m=2, w=W)

            def slot(i, p):
                return (i * NSLOT + p) * CHS

            def evict(p, bank, i):
                """PSUM bank (img i, pair p) -> fp16 stage, mask fused."""
                src = bank[:, 0:CHN].rearrange(
                    "p (h w) -> p h w", w=WP)[:, :, 0:W]
                dst = stage[:, slot(i, p):slot(i, p) + CHS].rearrange(
                    "p (h w) -> p h w", w=W)
                if p == 7:            # chunks (14,15): rows 44-47 masked
                    nc.vector.tensor_mul(dst, src, mk3[:, 0])
                elif 8 <= p <= 13:    # chunks (16..27): rows 48-83 masked
                    nc.vector.tensor_mul(dst, src, mk3[:, 1])
                elif i == 0:
                    nc.vector.tensor_copy(dst, src)
                else:
                    nc.scalar.copy(dst, src)

            def store_batch(s0, np_):
                """np_ pair-slots starting at slot s0, one DMA per image."""
                for i, eng in ((0, nc.sync), (1, nc.scalar)):
                    src = stage[:, slot(i, s0):slot(i, s0) + np_ * CHS]
                    dst = y_ap[i, 2 * s0:2 * s0 + 2 * np_, :, :].rearrange(
                        "(pr par) o f -> (par o) pr f", par=2)
                    eng.dma_start(out=dst,
                                  in_=src.rearrange("p (pr f) -> p pr f",
                                                    f=CHS))

            # PE warm-up: full-width (128x128) dummy matmuls on scratch SBUF
            # (stage slot written only much later) during the initial x-load
            # wait.  The HAM clock gate un-throttles only after a ~3.4us
            # window of *high-utilization* PE activity (64x64 single-quadrant
            # dummies measurably never flip it), and once warm it re-throttles
            # only after a fully idle window, so the dummies may end a couple
            # of microseconds before the real stream starts.
            dum = pp.tile([128, 512], f32, tag="ps", name="dum")
            scr = stage[:, slot(1, 20):slot(1, 20) + 512]
            for _ in range(40):
                nc.tensor.matmul(dum[:, 0:128], scr[:, 0:128],
                                 scr[:, 0:128], start=True, stop=True,
                                 skip_group_check=True)

            # matmul groups of 2 pairs (pair 0 alone so the stream can start
            # on the first 8-row x segment): consecutive matmuls within a
            # tap share the stationary weights
            groups = [(0,)] + [(2 * g + 1, 2 * g + 2) for g in range(10)]
            def leftover_block():
                """chunk 42 (rows 126-127): computed mid-stream (needs only
                the last x segment) so the kernel tail is short.  img0 in
                quadrant (r0, c0), img1 in quadrant (r1, c0): both land on
                PSUM partitions 0-63 and evict partition-aligned to slot 21."""
                n2 = 2 * WP  # 258
                pc_ = pp.tile([128, 512], f32, tag="ps", name="pc_")
                pd_ = pp.tile([128, 512], f32, tag="ps", name="pd_")
                for t, (kh, kw) in enumerate(TAPS):
                    st, sp = (t == 0), (t == len(TAPS) - 1)
                    nc.tensor.matmul(pc_[0:64, 0:n2], lhsT(0, t),
                                     rhs(0, NCH - 1, kh, kw, n2), start=st,
                                     stop=sp, skip_group_check=True)
                    nc.tensor.matmul(pd_[0:64, 0:n2], lhsT(1, t),
                                     rhs(1, NCH - 1, kh, kw, n2), start=st,
                                     stop=sp, skip_group_check=True)
                for i, bank in ((0, pc_), (1, pd_)):
                    src = bank[0:64, 0:n2].rearrange(
                        "p (h w) -> p h w", w=WP)[:, :, 0:W]
                    dst = stage[0:64,
                                slot(i, 21):slot(i, 21) + 2 * W].rearrange(
                        "p (h w) -> p h w", w=W)
                    if i == 0:
                        nc.vector.tensor_copy(dst, src)
                    else:
                        nc.scalar.copy(dst, src)

            for grp in groups:
                banks = {}
                for p in grp:
                    banks[p] = (pp.tile([128, 512], f32, tag="ps",
                                        name=f"pa{p}"),
                                pp.tile([128, 512], f32, tag="ps",
                                        name=f"pb{p}"))
                for t, (kh, kw) in enumerate(TAPS):
                    st, sp = (t == 0), (t == len(TAPS) - 1)
                    for half in (0, 1):   # img half: same lhsT across pairs
                        for p in grp:
                            bank = banks[p][half]
                            for c_par in (0, 1):
                                nc.tensor.matmul(
                                    bank[c_par * 64:(c_par + 1) * 64, 0:CHN],
                                    lhsT(half, t),
                                    rhs(half, 2 * p + c_par, kh, kw, CHN),
                                    start=st, stop=sp, skip_group_check=True)
                for p in grp:
                    evict(p, banks[p][0], 0)
                    evict(p, banks[p][1], 1)
                    if p in (3, 7, 11, 15):
                        store_batch(p - 3, 4)
                    elif p == 14:
                        leftover_block()
                    elif p == 18:
                        store_batch(16, 3)

            # final batch: pairs 19-20 + leftover slot (chunks 38-42 + pad)
            store_batch(19, 3)

    nc.compile()
    return nc


def _get_module():
    if "nc" not in _CACHE:
        _CACHE["nc"] = _build_module()
    return _CACHE["nc"]


def _build_mask():
    """[128, 768] fp32: [:, 0:384] = pair-7 mask (chunk 14 row 44 only in
    partitions 0-63, chunk 15 rows 45-47 in partitions 64-127); [:, 384:768]
    = full mask (all three rows) for pairs 8..13 (chunks 16..27)."""
    mk = np.ones((128, 2, RPC, W), dtype=np.float32)
    mk[:, 1, :, CROP0:CROP1] = 0.0          # full mask: every row
    mk[0:64, 0, 2, CROP0:CROP1] = 0.0       # pair 7, chunk 14: row 44 (j=2)
    mk[64:128, 0, :, CROP0:CROP1] = 0.0     # pair 7, chunk 15: rows 45-47
    return mk.reshape(128, 2 * CHS)


def _make_in_maps(x, weight):
    x = np.asarray(x, dtype=np.float32)
    weight = np.asarray(weight, dtype=np.float32)
    # host marshaling: pad x into the row-major stride-129 layout
    xp = np.zeros((B, C, HP, WP), dtype=np.float32)
    xp[:, :, 1:H + 1, 1:W + 1] = x
    xp = xp.reshape(B, C, XLEN)
    import ml_dtypes
    xp = xp.astype(ml_dtypes.bfloat16)
    # weight [oc, ic, kh, kw] -> [ic, (kh kw), oc], duplicated in both halves
    wt = np.ascontiguousarray(
        weight.transpose(1, 2, 3, 0).reshape(C, KS * KS * OC)
    ).astype(ml_dtypes.bfloat16)
    wt = np.concatenate([wt, wt], axis=0)  # [128, 576]
    mk = _build_mask()
    return [
        {"xin": np.ascontiguousarray(xp[k * IMGS:(k + 1) * IMGS]), "wt": wt,
         "mk": mk}
        for k in range(N_CORES)
    ]


def kernel(x, weight):
    from concourse.bass_utils import run_bass_kernel_spmd

    nc = _get_module()
    in_maps = _make_in_maps(x, weight)
    res = run_bass_kernel_spmd(nc, in_maps, list(range(N_CORES)))
    # host unshard: [2, 44, 64, 384] fp16 chunk-major -> [2, 64, 128, 128]
    outs = []
    for k in range(N_CORES):
        y = np.asarray(res.results[k]["yout"])  # [IMGS, NCHP, OC, CHS] fp16
        y = y.reshape(IMGS, NCHP, OC, RPC, W).transpose(0, 2, 1, 3, 4)
        y = y.reshape(IMGS, OC, NCHP * RPC, W)[:, :, :H, :]
        outs.append(y.astype(np.float32))
    return np.concatenate(outs, axis=0)
